# revision 38
# baseline (speedup 1.0000x reference)
"""Collaborative RNN (GRU-style user-state scan + big vocab projection) on 8 trn2 cores.

Strategy
--------
Data-parallel over batch: core c owns batch rows [4c, 4c+4) (512 (b,t) pairs).
Each core runs the scan for its rows and computes logits for its 512 output
rows over the FULL vocab -> [512, 30001]; host concatenates.

The scan is restructured by dependency *levels*: pair (b,t) depends only on the
previous occurrence of the same user in the same batch row.  With U=256 users
and S=128 steps most users appear 0-2 times, so the 128-step serial scan
collapses into ~5 fully-batched levels.  Level 0 (first occurrences) needs no
hidden-state input at all when h0 == 0 (the graded case).

Per-core index structure is passed as *data* (index vectors; one-hot
gather/scatter matrices are generated on device via iota + is_equal) so a
single SPMD program runs on all 8 cores.  The program itself only depends on
global level sizes.

Layouts: "T" tiles are [H=128 partitions, pairs in free dim]; "nat" tiles are
[pairs in partitions, H in free dim].  The gather matmul contracts pair chunks
of the natural state against on-device one-hots and yields h_prev directly in
transposed layout; embedding rows are accumulated into the r/z/c PSUMs with
transpose-matmuls, so the only explicit transpose per level is h_new back to
natural for the scatter matmul.
"""

import sys
import types

import numpy as np

# ---------------------------------------------------------------- constants
B, S, U, H, V = 32, 128, 256, 128, 30001
NC = 8
R = B // NC  # batch rows per core
N = R * S  # 512 output rows (pairs) per core
H2 = 2 * H
P = 128
NCH = N // P  # pair chunks per core
WS_CHUNK = 4096  # ws free-dim tile width (bf16)
VPAD = 30016  # padded logits row stride (64B-aligned bf16 store segments)
WS_PAD_W = [WS_CHUNK] * 7 + [VPAD - 7 * WS_CHUNK]  # SBUF tile widths (last 1344)
STG_W = 8192  # staging/store tile width (bf16, 2 MB per store)
STAGES = [(v0, min(STG_W, VPAD - v0)) for v0 in range(0, VPAD, STG_W)]
MM_N = 512  # moving free dim per matmul (one PSUM bank)

TRACE = False  # set by test.py for profiling runs
_LAST_RESULTS = {}  # test.py reads exec_time_ns etc. from here


def _install_ntff_hook():
    """Register the axon NTFF profiling hook (antenv.axon_hooks is a stub in
    this container).  Harmless if the .so lacks the profiling symbols."""
    try:
        import antenv

        if getattr(antenv, "axon_hooks", None) is not None:
            return
        mod = types.ModuleType("antenv.axon_hooks")
        mod._hook = None
        mod.set_axon_ntff_profile_hook = lambda h: setattr(mod, "_hook", h)
        mod.get_axon_ntff_profile_hook = lambda: mod._hook
        sys.modules["antenv.axon_hooks"] = mod
        antenv.axon_hooks = mod
        from trn_agent_boot.trn_boot import _ntff_profile_via_ctypes

        hook = _ntff_profile_via_ctypes("/opt/axon/libaxon_pjrt.so")
        if hook is not None:
            mod.set_axon_ntff_profile_hook(hook)
    except Exception:
        pass


# ---------------------------------------------------------------- host prep
def _fold(a, cols):
    """[cols*128] -> [128, cols] with column j = slice j*128:(j+1)*128."""
    return np.ascontiguousarray(a.reshape(cols, P).T)


def _levels_for_core(users_c):
    """occ/prev per flat pair index (p = r*S + t, natural order)."""
    occ = np.zeros(N, np.int32)
    prev = np.full(N, -1, np.int32)
    for r in range(R):
        seen_cnt = {}
        seen_last = {}
        row = users_c[r]
        for t in range(S):
            u = int(row[t])
            p = r * S + t
            occ[p] = seen_cnt.get(u, 0)
            prev[p] = seen_last.get(u, -1)
            seen_cnt[u] = occ[p] + 1
            seen_last[u] = p
    return occ, prev


def _aux_layout(kmax, nk, with_h0):
    """Column layout of the packed aux tensors (shared host/device).

    aux_i32 [P, iw]: items(NCH) | idx{k}(J_k) ... | h0_idx(NCH)?
    aux_f32 [P, fw]: per k: invm(NCH) | prev(n_k) | pk(J_k) | prevci(n_k)?
    """
    ioff = {"items": 0}
    ip = NCH
    foff = {}
    fp = 0
    for k in range(1, kmax):
        n = nk[k]
        J = (n + P - 1) // P
        ioff[f"idx{k}"] = ip
        ip += J
        foff[f"invm{k}"] = fp
        fp += NCH
        foff[f"prev{k}"] = fp
        fp += n
        foff[f"pk{k}"] = fp
        fp += J
        if k > 1:
            foff[f"prevci{k}"] = fp
            fp += n
    if with_h0:
        ioff["h0_idx"] = ip
        ip += NCH
    return ioff, ip, foff, fp


def _build_core_data(users, items, h0, with_h0):
    """Per-core level structure + global padded sizes."""
    cores = []
    kmax = 1
    for c in range(NC):
        occ, prev = _levels_for_core(users[c * R : (c + 1) * R])
        cores.append((occ, prev))
        kmax = max(kmax, int(occ.max()) + 1)

    nk = [0] * kmax
    for occ, _ in cores:
        for k in range(1, kmax):
            nk[k] = max(nk[k], int((occ == k).sum()))
    nk = [max(2, n) if k > 0 else 0 for k, n in enumerate(nk)]

    per_core = []
    for c in range(NC):
        occ, prev = cores[c]
        items_c = items[c * R : (c + 1) * R].reshape(-1).astype(np.int32)
        d = {"items_all": _fold(items_c, NCH)}
        if with_h0:
            users_c = users[c * R : (c + 1) * R].reshape(-1).astype(np.int32)
            local_r = np.repeat(np.arange(R, dtype=np.int32), S)
            d["h0_idx"] = _fold(local_r * U + users_c, NCH)
            d["h0c"] = np.ascontiguousarray(
                h0[c * R : (c + 1) * R].reshape(R * U, H), dtype=np.float32
            )
        for k in range(1, kmax):
            n = nk[k]
            J = (n + P - 1) // P
            pk = np.nonzero(occ == k)[0]
            prev_v = np.full(n, -1.0, np.float32)
            pk_v = np.full(J * P, -1.0, np.float32)
            idx_v = np.zeros(J * P, np.int32)
            invm = np.ones(N, np.float32)
            m = len(pk)
            prev_v[:m] = prev[pk]
            pk_v[:m] = pk
            idx_v[:m] = items_c[pk]
            invm[pk] = 0.0
            # prev indices replicated across partitions (comparand for is_equal)
            d[f"prev{k}"] = np.ascontiguousarray(
                np.broadcast_to(prev_v[None, :], (P, n))
            )
            if k > 1:
                # compact index of prev within level k-1's pair list
                prev_pk = np.nonzero(occ == k - 1)[0]
                pos = {int(p): i for i, p in enumerate(prev_pk)}
                ci = np.full(n, -1.0, np.float32)
                for i, p in enumerate(pk):
                    ci[i] = pos[int(prev[p])]
                d[f"prevci{k}"] = np.ascontiguousarray(
                    np.broadcast_to(ci[None, :], (P, n))
                )
            d[f"pk{k}"] = _fold(pk_v, J)
            d[f"idx{k}"] = _fold(idx_v, J)
            d[f"invm{k}"] = _fold(invm, NCH)
        per_core.append(d)

    # pack the small per-core aux tensors into two wide tensors (one DMA
    # each instead of ~20 queue-serialized small DMAs)
    ioff, iw, foff, fw = _aux_layout(kmax, nk, with_h0)
    packed = []
    for c in range(NC):
        d = per_core[c]
        ai = np.zeros((P, iw), np.int32)
        af = np.zeros((P, fw), np.float32)
        ai[:, 0:NCH] = d["items_all"]
        for k in range(1, kmax):
            J = (nk[k] + P - 1) // P
            ai[:, ioff[f"idx{k}"] : ioff[f"idx{k}"] + J] = d[f"idx{k}"]
            af[:, foff[f"invm{k}"] : foff[f"invm{k}"] + NCH] = d[f"invm{k}"]
            af[:, foff[f"prev{k}"] : foff[f"prev{k}"] + nk[k]] = d[f"prev{k}"]
            af[:, foff[f"pk{k}"] : foff[f"pk{k}"] + J] = d[f"pk{k}"]
            if k > 1:
                af[:, foff[f"prevci{k}"] : foff[f"prevci{k}"] + nk[k]] = d[
                    f"prevci{k}"
                ]
        if with_h0:
            ai[:, ioff["h0_idx"] : ioff["h0_idx"] + NCH] = d["h0_idx"]
        nd = {"aux_i32": ai, "aux_f32": af}
        if with_h0:
            nd["h0c"] = d["h0c"]
        packed.append(nd)
    return packed, kmax, nk


# ---------------------------------------------------------------- device build
def _build_program(kmax, nk, with_h0):
    import concourse.bacc as bacc
    import concourse.mybir as mybir
    import concourse.tile as tile
    from concourse import bass
    from concourse.masks import make_identity

    f32 = mybir.dt.float32
    bf16 = mybir.dt.bfloat16
    i32 = mybir.dt.int32
    AF = mybir.ActivationFunctionType
    OP = mybir.AluOpType

    nc = bacc.Bacc(None, target_bir_lowering=False)

    # ---- DRAM I/O
    ioff, iw, foff, fw = _aux_layout(kmax, nk, with_h0)
    aux_i32 = nc.dram_tensor("aux_i32", [P, iw], i32, kind="ExternalInput")
    aux_f32 = nc.dram_tensor("aux_f32", [P, fw], f32, kind="ExternalInput")
    P_cat = nc.dram_tensor("P_cat", [V, H2 + H], f32, kind="ExternalInput")
    if not with_h0:
        P_cat_bf = nc.dram_tensor("P_cat_bf", [V, H2 + H], bf16, kind="ExternalInput")
    WB = H2 + H + 3  # packed [W_ru | W_c | b_r | b_z | b_c]
    wpack = nc.dram_tensor("wpack", [H, WB], f32, kind="ExternalInput")
    ws = nc.dram_tensor("ws", [H, V], bf16, kind="ExternalInput")
    logits = nc.dram_tensor("logits", [N, VPAD], bf16, kind="ExternalOutput")
    if with_h0:
        h0c = nc.dram_tensor("h0c", [R * U, H], f32, kind="ExternalInput")

    with tile.TileContext(nc) as tc, tc.tile_pool(name="const", bufs=1) as cpool:
        with (
            tc.tile_pool(name="scan", bufs=2) as spool,
            tc.tile_pool(name="scan_ps", bufs=1, space="PSUM") as spsum,
        ):
            # ---- emission order matters: each engine queue executes in the
            # scheduled (roughly program) order.  The scan's critical-path
            # loads (items, level aux) go FIRST on the sync queue; the bulk
            # ws load is split across both HWDGE queues behind them.

            # packed aux loads first: they gate the whole scan
            ai_sb = cpool.tile([P, iw], i32, tag="ai_sb")
            nc.sync.dma_start(ai_sb[:], aux_i32[:])
            af_sb = cpool.tile([P, fw], f32, tag="af_sb")
            nc.sync.dma_start(af_sb[:], aux_f32[:])
            wb_sb = cpool.tile([H, WB], f32, tag="wb_sb")
            nc.scalar.dma_start(wb_sb[:], wpack[:])
            items_sb = ai_sb  # items at i32 offset 0
            w_ru0 = wb_sb[:, 0:H]
            w_ru1 = wb_sb[:, H:H2]
            w_c_ap = wb_sb[:, H2 : H2 + H]
            b_r_ap = wb_sb[:, WB - 3 : WB - 2]
            b_z_ap = wb_sb[:, WB - 2 : WB - 1]
            b_c_ap = wb_sb[:, WB - 1 : WB]

            # L0 embedding gathers head the gpsimd queue
            g_cat = []
            g_dt = f32 if with_h0 else bf16
            g_src = P_cat if with_h0 else P_cat_bf
            for c in range(NCH):
                t = spool.tile([P, H2 + H], g_dt, tag="g_cat", bufs=NCH, name="g_cat")
                nc.gpsimd.indirect_dma_start(
                    out=t[:],
                    out_offset=None,
                    in_=g_src[:],
                    in_offset=bass.IndirectOffsetOnAxis(
                        ap=items_sb[:, c : c + 1], axis=0
                    ),
                )
                g_cat.append(t)
            if with_h0:
                g_h0 = []
                for c in range(NCH):
                    g = spool.tile([P, H], f32, tag="g_h0", bufs=NCH, name="g_h0")
                    nc.gpsimd.indirect_dma_start(
                        out=g[:],
                        out_offset=None,
                        in_=h0c[:],
                        in_offset=bass.IndirectOffsetOnAxis(
                            ap=ai_sb[:, ioff["h0_idx"] + c : ioff["h0_idx"] + c + 1], axis=0
                        ),
                    )
                    g_h0.append(g)
            # per-level embedding gathers (prefetched; only need idx_sb)
            lvl_emb = {}
            for k in range(1, kmax):
                n = nk[k]
                J = (n + P - 1) // P
                ixo = ioff[f"idx{k}"]
                embs = []
                for j in range(J):
                    j0 = j * P
                    nj = min(P, n - j0)
                    e_cat = spool.tile(
                        [P, H2 + H], f32, tag="e_cat", bufs=2 * kmax, name="e_cat"
                    )
                    nc.gpsimd.indirect_dma_start(
                        out=e_cat[:nj, :],
                        out_offset=None,
                        in_=P_cat[:],
                        in_offset=bass.IndirectOffsetOnAxis(
                            ap=ai_sb[:nj, ixo + j : ixo + j + 1], axis=0
                        ),
                    )
                    embs.append(e_cat)
                lvl_emb[k] = embs

            # ws bulk load DELAYED behind the gathers: ws is only needed at
            # the drain (~64us) but its 7.7MB stream starves the shared DMA
            # engines and delays the small critical gathers by ~12us.  A
            # marker DMA depending on the last gather heads each HWDGE
            # queue; FIFO order holds the ws transfers back.
            last_g = lvl_emb[kmax - 1][-1] if kmax > 1 else g_cat[NCH - 1]
            mk = cpool.tile([1, 2], bf16, tag="mk")
            nc.vector.tensor_copy(mk[0:1, 0:2], last_g[0:1, 0:2])
            nc.sync.dma_start(logits[0:1, 0:2], mk[0:1, 0:2])
            nc.scalar.dma_start(logits[1:2, 0:2], mk[0:1, 0:2])
            ws_sb = []
            for i, wp in enumerate(WS_PAD_W):
                t = cpool.tile([H, wp], bf16, tag=f"ws{i}", name=f"ws{i}")
                w = min(wp, V - i * WS_CHUNK)
                q = nc.sync if i % 2 == 0 else nc.scalar
                q.dma_start(t[:, :w], ws[:, i * WS_CHUNK : i * WS_CHUNK + w])
                if w < wp:
                    nc.vector.memset(t[:, w:wp], 0.0)
                ws_sb.append(t)

            # helper tiles (gpsimd queue, after the gathers)
            ident = cpool.tile([P, P], f32, tag="ident")
            make_identity(nc, ident[:])
            ident_bf = cpool.tile([P, P], bf16, tag="ident_bf")
            nc.vector.tensor_copy(ident_bf[:], ident[:])
            iota_col_i = cpool.tile([P, NCH], i32, tag="iota_col_i")
            nc.gpsimd.iota(
                iota_col_i[:], pattern=[[P, NCH]], base=0, channel_multiplier=1
            )
            iota_col = cpool.tile([P, NCH], f32, tag="iota_col")
            nc.vector.tensor_copy(iota_col[:], iota_col_i[:])
            iota_row_i = cpool.tile([P, N], i32, tag="iota_row_i")
            nc.gpsimd.iota(
                iota_row_i[:], pattern=[[1, N]], base=0, channel_multiplier=0
            )
            iota_row = cpool.tile([P, N], f32, tag="iota_row")
            nc.vector.tensor_copy(iota_row[:], iota_row_i[:])

            # prewarm the Act engine's activation table group off the
            # critical path (saves the ~1.3us ACT_TABLE_LOAD before L0)
            warm = cpool.tile([P, 1], f32, tag="warm")
            nc.vector.memset(warm[:], 0.0)
            warm2 = cpool.tile([P, 1], f32, tag="warm2")
            nc.scalar.activation(warm2[:], warm[:], AF.Sigmoid)

            # one-hot gather/scatter matrices (DVE, all inputs ready early)
            lvl_sg = {}
            lvl_ss = {}
            for k in range(1, kmax):
                n = nk[k]
                J = (n + P - 1) // P
                pvo = foff[f"prev{k}"]
                pko = foff[f"pk{k}"]
                pco = foff.get(f"prevci{k}", 0)
                Jp = (nk[k - 1] + P - 1) // P if k > 1 else NCH
                sgs = {}
                sss = {}
                for j in range(J):
                    j0 = j * P
                    nj = min(P, n - j0)
                    for c in range(Jp):
                        sg_c = spool.tile(
                            [P, nj], f32, tag="sg_c", bufs=2 * NCH,
                            name="sg_c",
                        )
                        src_o = pvo if k == 1 else pco
                        nc.vector.tensor_scalar(
                            out=sg_c[:],
                            in0=af_sb[:, src_o + j0 : src_o + j0 + nj],
                            scalar1=iota_col[:, c : c + 1],
                            scalar2=None,
                            op0=OP.is_equal,
                        )
                        sgs[(j, c)] = sg_c
                    for c in range(NCH):
                        ss_c = spool.tile(
                            [P, P], f32, tag="ss_c", bufs=2 * NCH,
                            name="ss_c",
                        )
                        nc.vector.tensor_scalar(
                            out=ss_c[:nj, :],
                            in0=iota_row[:nj, c * P : (c + 1) * P],
                            scalar1=af_sb[:nj, pko + j : pko + j + 1],
                            scalar2=None,
                            op0=OP.is_equal,
                        )
                        sss[(j, c)] = ss_c
                lvl_sg[k] = sgs
                lvl_ss[k] = sss

            # persistent state
            h_nat = [
                cpool.tile([P, H], f32, tag=f"h_nat{c}", name=f"h_nat{c}")
                for c in range(NCH)
            ]
            hT = [
                cpool.tile([H, P], bf16, tag=f"hT{c}", name=f"hT{c}")
                for c in range(NCH)
            ]

            # ---------- level 0: all 512 pairs, full width, transposed layout
            zT = cpool.tile([H, N], f32, tag="zT")
            cT = cpool.tile([H, N], f32, tag="cT")
            hT0 = cpool.tile([H, N], f32, tag="hT0")
            l0_dt = f32 if with_h0 else bf16
            z_ps = spsum.tile([H, N], l0_dt, tag="z_ps2", name="z_ps")
            c_ps = spsum.tile([H, N], l0_dt, tag="c_ps2", name="c_ps")

            if not with_h0:
                # per-chunk pipeline: each chunk's z/c -> h -> h_nat flows as
                # soon as its gather lands, overlapping the remaining
                # gather transfers
                for c in range(NCH):
                    cs = slice(c * P, (c + 1) * P)
                    nc.tensor.matmul(
                        z_ps[:, cs],
                        g_cat[c][:, H:H2],
                        ident_bf[:],
                        is_transpose=True,
                        start=True,
                        stop=True,
                    )
                    nc.tensor.matmul(
                        c_ps[:, cs],
                        g_cat[c][:, H2 : H2 + H],
                        ident_bf[:],
                        is_transpose=True,
                        start=True,
                        stop=True,
                    )
                    nc.scalar.activation(
                        zT[:, cs], z_ps[:, cs], AF.Sigmoid, bias=b_z_ap
                    )
                    nc.scalar.activation(
                        cT[:, cs], c_ps[:, cs], AF.Tanh, bias=b_c_ap
                    )
                    # h = (1-z)*c = c - z*c
                    nc.vector.tensor_mul(hT0[:, cs], zT[:, cs], cT[:, cs])
                    nc.vector.tensor_sub(hT0[:, cs], cT[:, cs], hT0[:, cs])
                # transposes AFTER all chunks' matmuls: a transpose between
                # chunks would make the in-order PE wait on each chunk's
                # DVE chain before issuing the next chunk's z/c matmuls
                for c in range(NCH):
                    cs = slice(c * P, (c + 1) * P)
                    ps = spsum.tile([P, P], f32, tag="tr_ps", bufs=1, name="tr_ps")
                    nc.tensor.transpose(ps[:], hT0[:, cs], ident[:])
                    nc.vector.tensor_copy(h_nat[c][:], ps[:])
            else:
                hp_ps = spsum.tile([H, N], f32, tag="hp_ps", name="hp_ps")
                for c in range(NCH):
                    nc.tensor.matmul(
                        hp_ps[:, c * P : (c + 1) * P],
                        g_h0[c][:],
                        ident[:],
                        is_transpose=True,
                        start=(c == 0),
                        stop=(c == NCH - 1),
                    )
                hprevT = cpool.tile([H, N], f32, tag="hprevT0")
                nc.vector.tensor_copy(hprevT[:], hp_ps[:])

                r_ps = spsum.tile([H, N], f32, tag="r_ps", name="r_ps")
                for c in range(NCH):
                    nc.tensor.matmul(
                        r_ps[:, c * P : (c + 1) * P],
                        g_cat[c][:, 0:H],
                        ident[:],
                        is_transpose=True,
                        start=(c == 0),
                        stop=False,
                    )
                    nc.tensor.matmul(
                        z_ps[:, c * P : (c + 1) * P],
                        g_cat[c][:, H:H2],
                        ident[:],
                        is_transpose=True,
                        start=(c == 0),
                        stop=False,
                    )
                nc.tensor.matmul(
                    r_ps[:], w_ru0, hprevT[:], start=False, stop=True
                )
                nc.tensor.matmul(
                    z_ps[:], w_ru1, hprevT[:], start=False, stop=True
                )
                rT = cpool.tile([H, N], f32, tag="rT0")
                nc.scalar.activation(rT[:], r_ps[:], AF.Sigmoid, bias=b_r_ap)
                nc.scalar.activation(zT[:], z_ps[:], AF.Sigmoid, bias=b_z_ap)
                rh = cpool.tile([H, N], f32, tag="rh0")
                nc.vector.tensor_mul(rh[:], rT[:], hprevT[:])
                for c in range(NCH):
                    nc.tensor.matmul(
                        c_ps[:, c * P : (c + 1) * P],
                        g_cat[c][:, H2 : H2 + H],
                        ident[:],
                        is_transpose=True,
                        start=(c == 0),
                        stop=False,
                    )
                nc.tensor.matmul(c_ps[:], w_c_ap, rh[:], start=False, stop=True)
                nc.scalar.activation(cT[:], c_ps[:], AF.Tanh, bias=b_c_ap)
                # h = c + z*(hprev - c)
                nc.vector.tensor_sub(hT0[:], hprevT[:], cT[:])
                nc.vector.tensor_mul(hT0[:], zT[:], hT0[:])
                nc.vector.tensor_add(hT0[:], cT[:], hT0[:])

            if with_h0:
                # h_nat chunks (natural layout) from hT0
                for c in range(NCH):
                    ps = spsum.tile([P, P], f32, tag="tr_ps", bufs=1, name="tr_ps")
                    nc.tensor.transpose(
                        ps[:], hT0[:, c * P : (c + 1) * P], ident[:]
                    )
                    nc.vector.tensor_copy(h_nat[c][:], ps[:])

            # ---------- levels 1..kmax-1 (compact, padded size nk[k])
            hnew_prev = None
            for k in range(1, kmax):
                n = nk[k]
                J = (n + P - 1) // P
                imo = foff[f"invm{k}"]

                hnew_nat = []
                for j in range(J):
                    j0 = j * P
                    nj = min(P, n - j0)
                    e_cat = lvl_emb[k][j]
                    # gather h_prev directly in transposed layout [H, nj]:
                    # level 1 contracts the natural state chunks; deeper levels
                    # contract the PREVIOUS level's compact output tiles (their
                    # predecessors are level k-1 pairs by construction), which
                    # skips waiting for the scatter.
                    hp_ps = spsum.tile([H, P], f32, tag="hp_ps", name="hp_ps")
                    if k == 1:
                        for c in range(NCH):
                            nc.tensor.matmul(
                                hp_ps[:, :nj],
                                h_nat[c][:],
                                lvl_sg[k][(j, c)][:],
                                start=(c == 0),
                                stop=(c == NCH - 1),
                            )
                    else:
                        for ji, (hnp, njp, _) in enumerate(hnew_prev):
                            nc.tensor.matmul(
                                hp_ps[:, :nj],
                                hnp[:njp, :],
                                lvl_sg[k][(j, ji)][:njp, :],
                                start=(ji == 0),
                                stop=(ji == len(hnew_prev) - 1),
                            )
                    hprevT = spool.tile([H, P], f32, tag="hprevT", name="hprevT")
                    nc.vector.tensor_copy(hprevT[:, :nj], hp_ps[:, :nj])

                    # GRU math; embedding rows enter via transpose-matmuls
                    # (emitted first in each group so they run early)
                    r_ps = spsum.tile([H, P], f32, tag="r_ps", name="r_ps")
                    nc.tensor.matmul(
                        r_ps[:, :nj],
                        e_cat[:nj, 0:H],
                        ident[:nj, :nj],
                        is_transpose=True,
                        start=True,
                        stop=False,
                    )
                    nc.tensor.matmul(
                        r_ps[:, :nj],
                        w_ru0,
                        hprevT[:, :nj],
                        start=False,
                        stop=True,
                    )
                    rT = spool.tile([H, P], f32, tag="rT_l", name="rT")
                    nc.scalar.activation(
                        rT[:, :nj], r_ps[:, :nj], AF.Sigmoid, bias=b_r_ap
                    )
                    z_ps2 = spsum.tile([H, P], f32, tag="z_ps2", name="z_ps2")
                    nc.tensor.matmul(
                        z_ps2[:, :nj],
                        e_cat[:nj, H:H2],
                        ident[:nj, :nj],
                        is_transpose=True,
                        start=True,
                        stop=False,
                    )
                    nc.tensor.matmul(
                        z_ps2[:, :nj],
                        w_ru1,
                        hprevT[:, :nj],
                        start=False,
                        stop=True,
                    )
                    zTl = spool.tile([H, P], f32, tag="zT_l", name="zTl")
                    nc.scalar.activation(
                        zTl[:, :nj], z_ps2[:, :nj], AF.Sigmoid, bias=b_z_ap
                    )
                    rh = spool.tile([H, P], f32, tag="rh_l", name="rh")
                    nc.vector.tensor_mul(rh[:, :nj], rT[:, :nj], hprevT[:, :nj])
                    c_ps2 = spsum.tile([H, P], f32, tag="c_ps2", name="c_ps2")
                    nc.tensor.matmul(
                        c_ps2[:, :nj],
                        e_cat[:nj, H2 : H2 + H],
                        ident[:nj, :nj],
                        is_transpose=True,
                        start=True,
                        stop=False,
                    )
                    nc.tensor.matmul(
                        c_ps2[:, :nj],
                        w_c_ap,
                        rh[:, :nj],
                        start=False,
                        stop=True,
                    )
                    cTl = spool.tile([H, P], f32, tag="cT_l", name="cTl")
                    nc.scalar.activation(
                        cTl[:, :nj], c_ps2[:, :nj], AF.Tanh, bias=b_c_ap
                    )
                    # h_new = c + z*(hprev - c)
                    hnT = spool.tile([H, P], f32, tag="hnT_l", name="hnT")
                    nc.vector.tensor_sub(hnT[:, :nj], hprevT[:, :nj], cTl[:, :nj])
                    nc.vector.tensor_mul(hnT[:, :nj], zTl[:, :nj], hnT[:, :nj])
                    nc.vector.tensor_add(hnT[:, :nj], cTl[:, :nj], hnT[:, :nj])

                    hn = spool.tile([P, H], f32, tag="hn_nat", bufs=6, name="hn")
                    ps = spsum.tile([P, P], f32, tag="tr_ps", bufs=1, name="tr_ps")
                    nc.tensor.transpose(ps[:nj, :H], hnT[:, :nj], ident[:H, :H])
                    nc.vector.tensor_copy(hn[:nj, :], ps[:nj, :H])
                    hnew_nat.append((hn, nj, j0))

                # scatter back into h_nat (masked replace, fused update)
                for c in range(NCH):
                    d_ps = spsum.tile([P, H], f32, tag="d_ps", name="d_ps")
                    for ji, (hn, nj, j0) in enumerate(hnew_nat):
                        nc.tensor.matmul(
                            d_ps[:],
                            lvl_ss[k][(ji, c)][:nj, :],
                            hn[:nj, :],
                            start=(ji == 0),
                            stop=(ji == len(hnew_nat) - 1),
                        )
                    # h_nat = h_nat * invm + delta   (one DVE op)
                    nc.vector.scalar_tensor_tensor(
                        out=h_nat[c][:],
                        in0=h_nat[c][:],
                        scalar=af_sb[:, imo + c : imo + c + 1],
                        in1=d_ps[:],
                        op0=OP.mult,
                        op1=OP.add,
                    )
                hnew_prev = hnew_nat

            # ---------- final transposed state for the big matmul
            # own double-buffered PSUM tag: with tr_ps (bufs=1) each
            # transpose would wait the previous chunk's DVE copy
            for c in range(NCH):
                ps = spsum.tile([P, P], f32, tag="ftr", bufs=2, name="ftr")
                nc.tensor.transpose(ps[:], h_nat[c][:], ident[:])
                nc.vector.tensor_copy(hT[c][:], ps[:])


        # ---------- big projection: logits[128c : 128c+128, :] = hT[c].T @ ws
        # chunk-outer loop keeps the stationary weights (hT[c]) resident in
        # the PE array for ~59 consecutive matmuls so the PE ramps to full
        # clock; PSUM->SBUF copies alternate DVE/Act (casting f32->bf16) and
        # 2 MB stores alternate the two HWDGE queues.
        with (
            tc.tile_pool(name="big", bufs=4) as bpool,
            tc.tile_pool(name="big_ps", bufs=4, space="PSUM") as bpsum,
        ):
            cp = 0
            st = 0
            for c in range(NCH):
                # chunk 0's first stage split in half so the first store
                # issues after 4 copies instead of 8
                stages_c = (
                    [(0, STG_W // 2), (STG_W // 2, STG_W // 2)] + STAGES[1:]
                    if c == 0
                    else STAGES
                )
                for v0, sw in stages_c:
                    stage = bpool.tile([P, STG_W], bf16, tag="stage", name="stage")
                    # 512-wide sub-chunks over the padded ws tiles
                    subs = []
                    pos = 0
                    while pos < sw:
                        va = v0 + pos
                        wsi, off = divmod(va, WS_CHUNK)
                        w = min(MM_N, WS_PAD_W[wsi] - off, sw - pos)
                        subs.append((pos, wsi, off, w))
                        pos += w
                    i = 0
                    while i < len(subs):
                        grp = [subs[i]]
                        if i + 1 < len(subs) and subs[i][3] + subs[i + 1][3] <= 1024:
                            grp.append(subs[i + 1])
                        gw = sum(g[3] for g in grp)
                        o_ps = bpsum.tile([P, 1024], f32, tag="o_ps", name="o_ps")
                        gpos = 0
                        for _, wsi, off, w in grp:
                            nc.tensor.matmul(
                                o_ps[:, gpos : gpos + w],
                                hT[c][:],
                                ws_sb[wsi][:, off : off + w],
                                start=True,
                                stop=True,
                            )
                            gpos += w
                        sp0 = grp[0][0]
                        if cp % 2 == 0:
                            nc.vector.tensor_copy(
                                stage[:, sp0 : sp0 + gw], o_ps[:, :gw]
                            )
                        else:
                            nc.scalar.copy(stage[:, sp0 : sp0 + gw], o_ps[:, :gw])
                        cp += 1
                        i += len(grp)
                    q = nc.sync if st % 2 == 0 else nc.scalar
                    q.dma_start(
                        logits[c * P : (c + 1) * P, v0 : v0 + sw], stage[:, :sw]
                    )
                    st += 1

    nc.finalize()
    return nc


_PROGRAM_CACHE = {}


def kernel(users, items, h0, P_ru, W_ru, b_ru, P_c, W_c, b_c, ws):
    _install_ntff_hook()
    from concourse.bass_utils import run_bass_kernel_spmd

    users = np.asarray(users)
    items = np.asarray(items)
    h0 = np.asarray(h0, dtype=np.float32)
    with_h0 = bool(np.any(h0))

    per_core, kmax, nk = _build_core_data(users, items, h0, with_h0)

    key = (kmax, tuple(nk), with_h0)
    if key not in _PROGRAM_CACHE:
        _PROGRAM_CACHE[key] = _build_program(kmax, nk, with_h0)
    nc = _PROGRAM_CACHE[key]

    import ml_dtypes

    P_cat = np.concatenate(
        [np.asarray(P_ru, dtype=np.float32), np.asarray(P_c, dtype=np.float32)],
        axis=1,
    )
    wpack = np.concatenate(
        [
            np.asarray(W_ru, dtype=np.float32),
            np.asarray(W_c, dtype=np.float32),
            np.asarray(b_ru, dtype=np.float32)[0:H, None],
            np.asarray(b_ru, dtype=np.float32)[H:H2, None],
            np.asarray(b_c, dtype=np.float32)[:, None],
        ],
        axis=1,
    )
    shared = {
        "P_cat": P_cat,
        **(
            {}
            if with_h0
            else {"P_cat_bf": np.ascontiguousarray(P_cat.astype(ml_dtypes.bfloat16))}
        ),
        "wpack": np.ascontiguousarray(wpack),
        "ws": np.ascontiguousarray(
            np.asarray(ws, dtype=np.float32).astype(ml_dtypes.bfloat16)
        ),
    }
    in_maps = [{**shared, **per_core[c]} for c in range(NC)]

    res = run_bass_kernel_spmd(nc, in_maps, core_ids=list(range(NC)), trace=TRACE)
    _LAST_RESULTS["exec_time_ns"] = res.exec_time_ns
    _LAST_RESULTS["mean_exec_time_ns"] = res.mean_exec_time_ns
    _LAST_RESULTS["trace"] = res.instructions_and_trace
    _LAST_RESULTS["profile_json"] = res.profile_json

    out = np.empty((NC * N, V), np.float32)
    for c in range(NC):
        out[c * N : (c + 1) * N] = np.asarray(res.results[c]["logits"])[:, :V]
    return out

